# revision 3
# baseline (speedup 1.0000x reference)
"""Haar DWT edge-magnitude kernel for TRN2 (8 NeuronCores, SPMD) (PE-offloaded, software-pipelined).

out = sqrt(0.5*R[h,w]^2 + 0.5*R[h,w+1]^2 + 0.25*D[h,w]^2), reflect-padded
(so out row 255 == row 254 and col 255 == col 254).

Rows-on-partitions layout; the idle PE computes both vertical stencil
passes as float32r matmuls with bidiagonal stationary weights:
  R = Wr.T @ x          (Wr[m,m]=1, Wr[m+1,m]=-1)
  D = Wp.T @ Dh         (Wp scaled by 1/sqrt(2) so one ACT Square(scale)
                         pass evacuates R and D PSUM halves together)
DVE+GPSIMD split the horizontal diffs (Dh), the shifted sum s, and the
final add u; ACT does the fused square evacuation and the sqrt; out-DMAs
issue from the ACT queue right after their sqrt.  Chunks are 8 images
(2048 cols) except the first/last which are split in half to shorten
pipeline ramp and drain.  Software-pipelined emission:

  iter j: SP in(j+2) | GPS Dh_l(j), u_l(j-1)+fix | DVE Dh_r(j), s(j-1), u_r(j-1)
          | PE 4xmmR(j), 4xmmD(j) | ACT rq2(j), sqrt(j-2), out-dma(j-2)
"""

import numpy as np

import concourse.bass as bass
from concourse import bacc, mybir, tile
from concourse.bass_utils import run_bass_kernel_spmd

AF = mybir.ActivationFunctionType
OP = mybir.AluOpType
FP32 = mybir.dt.float32
F32R = mybir.dt.float32r

B, C, H, W = 8, 64, 256, 256
NCORES = 8
NIMG = (B * C) // NCORES        # 64 images per core
OP_ROWS = 127                    # output rows per row-tile
MMN = 512                        # matmul moving free size (1 PSUM bank)
FMAX = 2048

SQRT_HALF = float(np.sqrt(0.5))

# (h0, img0, nimg) chunks; first/last split for ramp/drain.
CHUNKS = []
for h0 in (0, 127):
    img_plan = [4, 4] + [8] * 7 if h0 == 0 else [8] * 7 + [4, 4]
    i0 = 0
    for n in img_plan:
        CHUNKS.append((h0, i0, n))
        i0 += n
NK = len(CHUNKS)


def build_nc(reps: int = 1):
    nc = bacc.Bacc("TRN2", target_bir_lowering=False)
    xd = nc.dram_tensor("x", [NIMG, H, W], F32R, kind="ExternalInput")
    wd = nc.dram_tensor("wts", [128, 2 * OP_ROWS], F32R, kind="ExternalInput")
    od = nc.dram_tensor("out", [NIMG, H, W], FP32, kind="ExternalOutput")

    with tile.TileContext(nc) as tc:
        with (
            tc.tile_pool(name="io", bufs=3) as io_pool,
            tc.tile_pool(name="tmp", bufs=3) as tmp_pool,
            tc.tile_pool(name="wp", bufs=1) as w_pool,
            tc.tile_pool(name="psr", bufs=1, space="PSUM") as psr_pool,
            tc.tile_pool(name="psd", bufs=1, space="PSUM") as psd_pool,
        ):
            wt = w_pool.tile([128, 2 * OP_ROWS], F32R, tag="wt")
            nc.sync.dma_start(wt[:], wd[:])
            wr = wt[:, 0:OP_ROWS]
            wp = wt[:, OP_ROWS : 2 * OP_ROWS]

            for _rep in range(reps):
                # ---- tail: out rows 254 (==255) from in rows 254/255 ----
                tt = io_pool.tile([NIMG, 2 * W], F32R, tag="tin", bufs=1)
                ttv = tt.bitcast(FP32)
                nc.sync.dma_start(tt[:], xd[:, H - 2 : H, :])
                rt = tmp_pool.tile([NIMG, W], FP32, tag="trt", bufs=1)
                nc.vector.tensor_tensor(
                    rt[:], ttv[:, 0:W], ttv[:, W : 2 * W], OP.subtract
                )
                pt = tmp_pool.tile([NIMG, W], FP32, tag="tpt", bufs=1)
                nc.gpsimd.tensor_tensor(
                    pt[:], ttv[:, 0:W], ttv[:, W : 2 * W], OP.add
                )
                r2t = tmp_pool.tile([NIMG, W], FP32, tag="tr2", bufs=1)
                nc.vector.scalar_tensor_tensor(
                    r2t[:], rt[:], 0.5, rt[:], OP.mult, OP.mult
                )
                st = tmp_pool.tile([NIMG, W], FP32, tag="tst", bufs=1)
                nc.vector.tensor_tensor(
                    st[:, 0 : W - 1], r2t[:, 0 : W - 1], r2t[:, 1:W], OP.add
                )
                dtt = tmp_pool.tile([NIMG, W], FP32, tag="tdt", bufs=1)
                nc.vector.tensor_tensor(
                    dtt[:, 0 : W - 1], pt[:, 0 : W - 1], pt[:, 1:W], OP.subtract
                )
                q2t = tmp_pool.tile([NIMG, W], FP32, tag="tq2", bufs=1)
                nc.vector.scalar_tensor_tensor(
                    q2t[:, 0 : W - 1],
                    dtt[:, 0 : W - 1],
                    0.25,
                    dtt[:, 0 : W - 1],
                    OP.mult,
                    OP.mult,
                )
                ut = tmp_pool.tile([NIMG, W], FP32, tag="tut", bufs=1)
                nc.gpsimd.tensor_tensor(
                    ut[:, 0 : W - 1], st[:, 0 : W - 1], q2t[:, 0 : W - 1], OP.add
                )
                nc.gpsimd.tensor_copy(ut[:, W - 1 : W], ut[:, W - 2 : W - 1])
                ot = io_pool.tile([NIMG, W], FP32, tag="tout", bufs=1)
                nc.scalar.activation(ot[:], ut[:], AF.Sqrt)
                nc.scalar.dma_start(od[:, H - 2, :], ot[:])
                nc.scalar.dma_start(od[:, H - 1, :], ot[:])

                # ---- software-pipelined main loop over CHUNKS ----
                xt = {}
                rq = {}
                s = {}
                u = {}
                psd_live = {}

                def F_of(k):
                    return CHUNKS[k][2] * W

                def dma_in(k):
                    h0, i0, n = CHUNKS[k]
                    xt[k] = io_pool.tile(
                        [128, FMAX], F32R, tag="in", name="xt", bufs=4
                    )
                    nc.sync.dma_start(
                        xt[k][:, 0 : n * W],
                        xd[i0 : i0 + n, h0 : h0 + 128, :].transpose([1, 0, 2]),
                    )

                def stage_front(k):
                    F = F_of(k)
                    xv = xt[k].bitcast(FP32)
                    dh = tmp_pool.tile([128, FMAX], F32R, tag="dh")
                    dsp = F // 2
                    nc.gpsimd.tensor_tensor(
                        dh[:, 0:dsp],
                        xv[:, 0:dsp],
                        xv[:, 1 : dsp + 1],
                        OP.subtract,
                    )
                    nc.vector.tensor_tensor(
                        dh[:, dsp : F - 1],
                        xv[:, dsp : F - 1],
                        xv[:, dsp + 1 : F],
                        OP.subtract,
                    )
                    nc.vector.tensor_tensor(
                        dh[:, F - 1 : F],
                        xv[:, F - 1 : F],
                        xv[:, F - 1 : F],
                        OP.subtract,
                    )
                    psr = psr_pool.tile([OP_ROWS, FMAX], FP32, tag="psr")
                    for m in range(F // MMN):
                        nc.tensor.matmul(
                            psr[:, m * MMN : (m + 1) * MMN],
                            wr,
                            xt[k][:, m * MMN : (m + 1) * MMN],
                            start=True,
                            stop=True,
                        )
                    psd = psd_pool.tile([OP_ROWS, FMAX], FP32, tag="psd")
                    for m in range(F // MMN):
                        nc.tensor.matmul(
                            psd[:, m * MMN : (m + 1) * MMN],
                            wp,
                            dh[:, m * MMN : (m + 1) * MMN],
                            start=True,
                            stop=True,
                        )
                    # rq[k][:, 0:F] = 0.5*R^2 ; [:, FMAX:FMAX+F] = 0.25*D^2
                    rq[k] = tmp_pool.tile(
                        [OP_ROWS, 2 * FMAX], FP32, tag="rq", name="rq"
                    )
                    nc.scalar.activation(
                        rq[k][:, 0:F], psr[:, 0:F], AF.Square, scale=SQRT_HALF
                    )
                    qsp = F // 4
                    nc.scalar.activation(
                        rq[k][:, FMAX + qsp : FMAX + F], psd[:, qsp:F], AF.Square,
                        scale=SQRT_HALF,
                    )
                    psd_live[k] = psd

                def stage_s(k):
                    F = F_of(k)
                    s[k] = tmp_pool.tile([OP_ROWS, FMAX], FP32, tag="s", name="s")
                    nc.vector.tensor_tensor(
                        s[k][:, 0 : F - 1],
                        rq[k][:, 0 : F - 1],
                        rq[k][:, 1:F],
                        OP.add,
                    )

                def stage_u(k):
                    F = F_of(k)
                    usp = F // 2
                    u[k] = tmp_pool.tile([OP_ROWS, FMAX], FP32, tag="u", name="u")
                    nc.gpsimd.tensor_tensor(
                        u[k][:, 0:usp],
                        s[k][:, 0:usp],
                        rq[k][:, FMAX : FMAX + usp],
                        OP.add,
                    )
                    nc.vector.tensor_tensor(
                        u[k][:, usp : F - 1],
                        s[k][:, usp : F - 1],
                        rq[k][:, FMAX + usp : FMAX + F - 1],
                        OP.add,
                    )
                    # reflect right edge: col 255 of each image = col 254
                    nc.gpsimd.tensor_copy(
                        u[k][:, W - 1 : F : W], u[k][:, W - 2 : F : W]
                    )
                    del s[k], rq[k]

                def stage_q2shed(k):
                    # q2 cols [0:F//4]: DVE evacuates psd, GPSIMD squares
                    F = F_of(k)
                    qsp = F // 4
                    dsb = tmp_pool.tile([OP_ROWS, FMAX // 4], FP32, tag="dsb")
                    # dsb = D/2 (psd holds D/sqrt(2)); q2 = dsb*dsb = D^2/4
                    nc.vector.tensor_scalar_mul(
                        dsb[:, 0:qsp], psd_live[k][:, 0:qsp], SQRT_HALF
                    )
                    nc.gpsimd.tensor_tensor(
                        rq[k][:, FMAX : FMAX + qsp],
                        dsb[:, 0:qsp],
                        dsb[:, 0:qsp],
                        OP.mult,
                    )
                    del psd_live[k]

                def stage_out(k):
                    F = F_of(k)
                    h0, i0, n = CHUNKS[k]
                    o = io_pool.tile([OP_ROWS, FMAX], FP32, tag="out", name="o")
                    nc.scalar.activation(o[:, 0:F], u[k][:, 0:F], AF.Sqrt)
                    oh0 = 0 if h0 == 0 else 127
                    nc.scalar.dma_start(
                        od[i0 : i0 + n, oh0 : oh0 + OP_ROWS, :].transpose(
                            [1, 0, 2]
                        ),
                        o[:, 0:F],
                    )
                    del u[k], xt[k]

                dma_in(0)
                for k in range(NK):
                    if k + 1 < NK:
                        dma_in(k + 1)
                    stage_front(k)
                    if k >= 1:
                        stage_s(k - 1)
                        stage_u(k - 1)
                    stage_q2shed(k)
                    if k >= 2:
                        stage_out(k - 2)
                stage_s(NK - 1)
                stage_u(NK - 1)
                stage_out(NK - 2)
                stage_out(NK - 1)
    nc.compile()
    return nc


def make_weights() -> np.ndarray:
    w = np.zeros((128, 2 * OP_ROWS), dtype=np.float32)
    for m in range(OP_ROWS):
        w[m, m] = 1.0
        w[m + 1, m] = -1.0
        # D weights pre-scaled by 1/sqrt(2): Square(sqrt(.5)*D') = 0.25*D^2
        w[m, OP_ROWS + m] = SQRT_HALF
        w[m + 1, OP_ROWS + m] = SQRT_HALF
    return w


def shard_input(x: np.ndarray) -> list[np.ndarray]:
    xr = np.ascontiguousarray(x, dtype=np.float32).reshape(B * C, H, W)
    return [
        np.ascontiguousarray(xr[i * NIMG : (i + 1) * NIMG]) for i in range(NCORES)
    ]


def unshard_output(outs: list[np.ndarray]) -> np.ndarray:
    full = np.empty((B * C, H, W), dtype=np.float32)
    for i, o in enumerate(outs):
        full[i * NIMG : (i + 1) * NIMG] = o
    return full.reshape(B, C, H, W)


def kernel(x: np.ndarray) -> np.ndarray:
    nc = build_nc()
    w = make_weights()
    in_maps = [{"x": s, "wts": w} for s in shard_input(x)]
    res = run_bass_kernel_spmd(nc, in_maps, core_ids=list(range(NCORES)))
    return unshard_output([r["out"] for r in res.results])
